# revision 1
# baseline (speedup 1.0000x reference)
"""Distributed Trainium2 kernel for nn_AdaptiveAvgPoolSequence.

Computation (reference): bucketize N=65536 points into an 8x8 spatial grid,
take the per-bin mean of values [B, N, C] over the point axis, flatten to
[B, 64*C], then a Linear to [B, 512].

Sharding across 8 NeuronCores:
  - points axis N split 8 ways (segment-sum is order/partition invariant)
  - each core computes partial per-bin sums [64, B*C] + counts via a
    one-hot matmul on the TensorEngine, accumulated in PSUM
  - ReduceScatter(add) gives each core the complete sums for its 8 bins
  - W is row-sharded [2048, 512] per core to match those 8 bins; each core
    computes a partial output [B, 512], AllReduce(add) finishes the Linear
"""

import numpy as np

import concourse.bacc as bacc
import concourse.mybir as mybir
import concourse.tile as tile
from concourse.bass_utils import run_bass_kernel_spmd

N_CORES = 8
B, N, C = 4, 65536, 256
NS = N // N_CORES          # 8192 points per core
J = NS // 128              # 64 contraction chunks of 128 points
HW = 64                    # 8x8 bins
HB = HW // N_CORES         # 8 bins owned per core after ReduceScatter
KK = HB * C // 128         # 16 K-chunks of the per-core Linear contraction
OUT = 512
BC = B * C                 # 1024

# Bin edges Tx[1..8] == Ty[1..8] of jnp.linspace(-1-1e-6, 1+1e-6, 9) in
# float32, hardcoded as bit patterns so device comparisons match the
# reference searchsorted bit-for-bit.
_EDGE_BITS = np.array(
    [3208642572, 3204448264, 3196059656, 0,
     1048576008, 1056964616, 1061158924, 1065353224],
    dtype=np.uint32,
)
EDGES = _EDGE_BITS.view(np.float32)

_NC = None


def _build():
    f32 = mybir.dt.float32
    add = mybir.AluOpType.add
    is_ge = mybir.AluOpType.is_ge
    is_eq = mybir.AluOpType.is_equal
    mult = mybir.AluOpType.mult
    amax = mybir.AluOpType.max
    cores = list(range(N_CORES))

    nc = bacc.Bacc("TRN2", debug=False, num_devices=N_CORES)
    values = nc.dram_tensor("values", [B, NS, C], f32, kind="ExternalInput")
    coords = nc.dram_tensor("coords", [NS, 2], f32, kind="ExternalInput")
    w_ext = nc.dram_tensor("W", [HB * C, OUT], f32, kind="ExternalInput")
    b_ext = nc.dram_tensor("b", [OUT], f32, kind="ExternalInput")
    out_ext = nc.dram_tensor("out", [B, OUT], f32, kind="ExternalOutput")

    rs_in = nc.dram_tensor("rs_in", [HW, BC + 1], f32)
    rs_out = nc.dram_tensor("rs_out", [HB, BC + 1], f32)
    ar_in = nc.dram_tensor("ar_in", [B, OUT], f32)
    ar_out = nc.dram_tensor("ar_out", [B, OUT], f32)

    with tile.TileContext(nc) as tc:
        with (
            tc.tile_pool(name="const", bufs=1) as cp,
            tc.tile_pool(name="vp", bufs=8) as vp,
            tc.tile_pool(name="ohp", bufs=4) as ohp,
            tc.tile_pool(name="sb", bufs=1) as sb,
            tc.tile_pool(name="pp", bufs=1, space="PSUM") as pp,
            tc.tile_pool(name="ppt", bufs=2, space="PSUM") as ppt,
        ):
            # ---- constants / small inputs ----
            w_sb = cp.tile([128, KK, OUT], f32)
            nc.sync.dma_start(w_sb[:], w_ext.ap().rearrange("(kk p) o -> p kk o", p=128))
            b_sb = cp.tile([1, OUT], f32)
            nc.sync.dma_start(b_sb[:], b_ext.ap().unsqueeze(0))
            ctile = cp.tile([128, 2 * J], f32)
            nc.sync.dma_start(ctile[:], coords.ap().rearrange("(p j) s -> p (j s)", p=128))

            iota64 = cp.tile([128, HW], f32)
            nc.gpsimd.iota(iota64[:], pattern=[[1, HW]], base=0,
                           channel_multiplier=0, allow_small_or_imprecise_dtypes=True)
            iota_p = cp.tile([128, 1], f32)
            nc.gpsimd.iota(iota_p[:], pattern=[[0, 1]], base=0,
                           channel_multiplier=1, allow_small_or_imprecise_dtypes=True)
            ones128 = cp.tile([128, 1], f32)
            nc.vector.memset(ones128[:], 1.0)
            ones_b = cp.tile([1, B], f32)
            nc.vector.memset(ones_b[:], 1.0 / N_CORES)
            id8 = cp.tile([8, 8], f32)
            nc.vector.tensor_scalar(id8[:], iota64[0:8, 0:8], iota_p[0:8, :], None, is_eq)

            # ---- per-point bin ids, [128, J]; point n = p*J + j ----
            cview = ctile[:].rearrange("p (j s) -> p j s", s=2)
            x_ap, y_ap = cview[:, :, 0], cview[:, :, 1]
            binsf = sb.tile([128, J], f32)
            ybins = sb.tile([128, J], f32)
            nc.vector.tensor_scalar(binsf[:], x_ap, float(EDGES[0]), None, is_ge)
            for e in EDGES[1:]:
                nc.vector.scalar_tensor_tensor(binsf[:], x_ap, float(e), binsf[:], is_ge, add)
            nc.vector.tensor_scalar(ybins[:], y_ap, float(EDGES[0]), None, is_ge)
            for e in EDGES[1:]:
                nc.vector.scalar_tensor_tensor(ybins[:], y_ap, float(e), ybins[:], is_ge, add)
            # bins = x_bins + 8 * y_bins
            nc.vector.scalar_tensor_tensor(binsf[:], ybins[:], 8.0, binsf[:], mult, add)

            # ---- segment sums via one-hot matmul, accumulated in PSUM ----
            psum_a = pp.tile([HW, 512], f32)
            psum_b = pp.tile([HW, 512], f32)
            psum_c = pp.tile([HW, 1], f32)
            vre = values.ap().rearrange("b (p j) c -> p j b c", p=128)
            for j in range(J):
                vt = vp.tile([128, BC], f32)
                nc.sync.dma_start(vt[:].rearrange("p (b c) -> p b c", b=B), vre[:, j])
                oh = ohp.tile([128, HW], f32)
                nc.vector.tensor_scalar(oh[:], iota64[:], binsf[:, j:j + 1], None, is_eq)
                st, sp = (j == 0), (j == J - 1)
                nc.tensor.matmul(psum_a[:], oh[:], vt[:, 0:512], start=st, stop=sp)
                nc.tensor.matmul(psum_b[:], oh[:], vt[:, 512:1024], start=st, stop=sp)
                nc.tensor.matmul(psum_c[:], oh[:], ones128[:], start=st, stop=sp)

            sums_sb = sb.tile([HW, BC + 1], f32)
            nc.any.tensor_copy(sums_sb[:, 0:512], psum_a[:])
            nc.any.tensor_copy(sums_sb[:, 512:1024], psum_b[:])
            nc.any.tensor_copy(sums_sb[:, 1024:1025], psum_c[:])

            # ---- reduce partial sums across cores; core i keeps bins [8i, 8i+8) ----
            nc.sync.dma_start(rs_in.ap(), sums_sb[:])
            nc.gpsimd.collective_compute(
                "ReduceScatter", add, replica_groups=[cores],
                ins=[rs_in.ap().opt()], outs=[rs_out.ap().opt()],
            )
            red = sb.tile([HB, BC + 1], f32)
            nc.sync.dma_start(red[:], rs_out.ap())

            # ---- means for the owned 8 bins ----
            cnt = sb.tile([HB, 1], f32)
            nc.vector.tensor_scalar(cnt[:], red[:, BC:BC + 1], 1.0, None, amax)
            rec = sb.tile([HB, 1], f32)
            nc.vector.reciprocal(rec[:], cnt[:])
            means = sb.tile([HB, BC], f32)
            nc.vector.tensor_scalar(means[:], red[:, 0:BC], rec[:], None, mult)

            # ---- transpose means into Linear lhsT layout ----
            # lhst[:, kk*4+b4] column p holds means[h, b4*256 + ch*128 + p]
            # for kk = h*2 + ch; matches W rows kk*128 .. kk*128+128.
            lhst = sb.tile([128, KK * B], f32)
            for b4 in range(B):
                for ch in range(2):
                    pt = ppt.tile([128, HB], f32)
                    lo = b4 * C + ch * 128
                    nc.tensor.transpose(pt[:], means[:, lo:lo + 128], id8[:])
                    dst = lhst[:].rearrange("p (h q) -> p h q", q=8)[:, :, 4 * ch + b4]
                    nc.any.tensor_copy(dst, pt[:])

            # ---- per-core partial Linear + bias/8, then AllReduce ----
            psum_o = pp.tile([B, OUT], f32)
            for kk in range(KK):
                nc.tensor.matmul(psum_o[:], lhst[:, kk * B:(kk + 1) * B],
                                 w_sb[:, kk, :], start=(kk == 0), stop=False)
            nc.tensor.matmul(psum_o[:], ones_b[:], b_sb[:], start=False, stop=True)
            out_sb = sb.tile([B, OUT], f32)
            nc.any.tensor_copy(out_sb[:], psum_o[:])
            nc.sync.dma_start(ar_in.ap(), out_sb[:])
            nc.gpsimd.collective_compute(
                "AllReduce", add, replica_groups=[cores],
                ins=[ar_in.ap().opt()], outs=[ar_out.ap().opt()],
            )
            nc.sync.dma_start(out_ext.ap(), ar_out.ap())

    nc.compile()
    return nc


def _get_nc():
    global _NC
    if _NC is None:
        _NC = _build()
    return _NC


def _shard(values, coords, W, b):
    values = np.ascontiguousarray(values, dtype=np.float32)
    coords = np.ascontiguousarray(coords, dtype=np.float32)
    W = np.ascontiguousarray(W, dtype=np.float32)
    b = np.ascontiguousarray(b, dtype=np.float32)
    in_maps = []
    for i in range(N_CORES):
        in_maps.append({
            "values": np.ascontiguousarray(values[:, i * NS:(i + 1) * NS, :]),
            "coords": np.ascontiguousarray(coords[i * NS:(i + 1) * NS]),
            "W": np.ascontiguousarray(W[i * HB * C:(i + 1) * HB * C]),
            "b": b,
        })
    return in_maps


def kernel(values, coords, W, b):
    nc = _get_nc()
    in_maps = _shard(values, coords, W, b)
    res = run_bass_kernel_spmd(nc, in_maps, core_ids=list(range(N_CORES)))
    return np.asarray(res.results[0]["out"], dtype=np.float32)


# revision 5
# speedup vs baseline: 1.4811x; 1.4811x over previous
"""Distributed Trainium2 kernel for nn_AdaptiveAvgPoolSequence.

Computation (reference): bucketize N=65536 points into an 8x8 spatial grid,
take the per-bin mean of values [B, N, C] over the point axis, flatten to
[B, 64*C], then a Linear to [B, 512].

Sharding across 8 NeuronCores:
  - points axis N split 8 ways (segment-sum is order/partition invariant)
  - each core computes partial per-bin sums [64, B*C] (one-hot matmul on
    the TensorEngine in bf16, accumulated in fp32 PSUM) + counts (one-hot
    accumulation on the vector engine)
  - AllToAll exchanges per-bin partial sums so core i holds all 8 cores'
    partials for bins [8i, 8i+8); a small matmul reduces them
  - W is row-sharded [2048, 512] per core to match those 8 bins; each core
    computes a partial output [B, 512]; AllReduce(add) finishes the Linear
"""

import numpy as np

import concourse.bacc as bacc
import concourse.mybir as mybir
import concourse.tile as tile
from concourse.bass_utils import run_bass_kernel_spmd

N_CORES = 8
B, N, C = 4, 65536, 256
NS = N // N_CORES          # 8192 points per core
J = NS // 128              # 64 contraction chunks of 128 points
HW = 64                    # 8x8 bins
HB = HW // N_CORES         # 8 bins owned per core after the exchange
KK = HB * C // 128         # 16 K-chunks of the per-core Linear contraction
OUT = 512
BC = B * C                 # 1024

# Bin edges Tx[1..8] == Ty[1..8] of jnp.linspace(-1-1e-6, 1+1e-6, 9) in
# float32, hardcoded as bit patterns so device comparisons match the
# reference searchsorted bit-for-bit.
_EDGE_BITS = np.array(
    [3208642572, 3204448264, 3196059656, 0,
     1048576008, 1056964616, 1061158924, 1065353224],
    dtype=np.uint32,
)
EDGES = _EDGE_BITS.view(np.float32)

_NC = None


def _build():
    f32 = mybir.dt.float32
    bf16 = mybir.dt.bfloat16
    add = mybir.AluOpType.add
    is_ge = mybir.AluOpType.is_ge
    is_eq = mybir.AluOpType.is_equal
    mult = mybir.AluOpType.mult
    amax = mybir.AluOpType.max
    cores = list(range(N_CORES))

    nc = bacc.Bacc("TRN2", debug=False, num_devices=N_CORES)
    values = nc.dram_tensor("values", [B, NS, C], f32, kind="ExternalInput")
    coords = nc.dram_tensor("coords", [NS, 2], f32, kind="ExternalInput")
    w_ext = nc.dram_tensor("W", [HB * C, OUT], f32, kind="ExternalInput")
    b_ext = nc.dram_tensor("b", [OUT], f32, kind="ExternalInput")
    out_ext = nc.dram_tensor("out", [B, OUT], f32, kind="ExternalOutput")

    a2a_in = nc.dram_tensor("a2a_in", [HW, BC + 1], f32)
    a2a_out = nc.dram_tensor("a2a_out", [HW, BC + 1], f32)
    ar_in = nc.dram_tensor("ar_in", [B, OUT], f32)
    ar_out = nc.dram_tensor("ar_out", [B, OUT], f32)

    # sel[p, h] = (p % 8 == h): reduces the 8 interleaved partials after A2A
    sel_np = (np.arange(HW)[:, None] % HB == np.arange(HB)[None, :]).astype(np.float32)
    sel_ext = nc.inline_tensor(sel_np, name="selmat")

    with tile.TileContext(nc) as tc:
        with (
            tc.tile_pool(name="const", bufs=1) as cp,
            tc.tile_pool(name="vp", bufs=6) as vp,
            tc.tile_pool(name="vbp", bufs=6) as vbp,
            tc.tile_pool(name="ohp", bufs=4) as ohp,
            tc.tile_pool(name="sb", bufs=1) as sb,
            tc.tile_pool(name="pp", bufs=1, space="PSUM") as pp,
            tc.tile_pool(name="ppt", bufs=2, space="PSUM") as ppt,
        ):
            # ---- constants / small inputs ----
            w_sb = cp.tile([128, KK, OUT], f32)
            nc.sync.dma_start(w_sb[:], w_ext.ap().rearrange("(kk p) o -> p kk o", p=128))
            w_bf = cp.tile([128, KK * OUT], bf16)
            nc.vector.tensor_copy(w_bf[:], w_sb[:].rearrange("p kk o -> p (kk o)"))
            b_sb = cp.tile([1, OUT], f32)
            nc.sync.dma_start(b_sb[:], b_ext.ap().unsqueeze(0))
            ctile = cp.tile([128, 2 * J], f32)
            nc.sync.dma_start(ctile[:], coords.ap().rearrange("(p j) s -> p (j s)", p=128))
            sel_sb = cp.tile([HW, HB], f32)
            nc.sync.dma_start(sel_sb[:], sel_ext.ap())

            iota64 = cp.tile([128, HW], f32)
            nc.gpsimd.iota(iota64[:], pattern=[[1, HW]], base=0,
                           channel_multiplier=0, allow_small_or_imprecise_dtypes=True)
            iota_p = cp.tile([128, 1], f32)
            nc.gpsimd.iota(iota_p[:], pattern=[[0, 1]], base=0,
                           channel_multiplier=1, allow_small_or_imprecise_dtypes=True)
            ones_b = cp.tile([1, B], f32)
            nc.vector.memset(ones_b[:], 1.0 / N_CORES)
            id8 = cp.tile([8, 8], f32)
            nc.vector.tensor_scalar(id8[:], iota64[0:8, 0:8], iota_p[0:8, :], None, is_eq)

            # ---- per-point bin ids, [128, J]; point n = p*J + j ----
            cview = ctile[:].rearrange("p (j s) -> p j s", s=2)
            x_ap, y_ap = cview[:, :, 0], cview[:, :, 1]
            binsf = sb.tile([128, J], f32)
            ybins = sb.tile([128, J], f32)
            nc.vector.tensor_scalar(binsf[:], x_ap, float(EDGES[0]), None, is_ge)
            for e in EDGES[1:]:
                nc.vector.scalar_tensor_tensor(binsf[:], x_ap, float(e), binsf[:], is_ge, add)
            nc.vector.tensor_scalar(ybins[:], y_ap, float(EDGES[0]), None, is_ge)
            for e in EDGES[1:]:
                nc.vector.scalar_tensor_tensor(ybins[:], y_ap, float(e), ybins[:], is_ge, add)
            # bins = x_bins + 8 * y_bins
            nc.vector.scalar_tensor_tensor(binsf[:], ybins[:], 8.0, binsf[:], mult, add)

            # ---- segment sums via one-hot matmul (bf16), accumulated in PSUM ----
            psum_a = pp.tile([HW, 512], f32, tag="pa")
            psum_b = pp.tile([HW, 512], f32, tag="pb")
            cnt_acc = sb.tile([128, HW], f32)
            nc.vector.memset(cnt_acc[:], 0.0)
            vre = values.ap().rearrange("b (p j) c -> p j b c", p=128)
            for j in range(J):
                vt = vp.tile([128, BC], f32)
                nc.sync.dma_start(vt[:].rearrange("p (b c) -> p b c", b=B), vre[:, j])
                vb = vbp.tile([128, BC], bf16)
                nc.vector.tensor_copy(vb[:], vt[:])
                oh = ohp.tile([128, HW], bf16)
                nc.vector.tensor_scalar(oh[:], iota64[:], binsf[:, j:j + 1], None, is_eq)
                nc.vector.tensor_tensor(cnt_acc[:], cnt_acc[:], oh[:], add)
                st, sp = (j == 0), (j == J - 1)
                nc.tensor.matmul(psum_a[:], oh[:], vb[:, 0:512], start=st, stop=sp)
                nc.tensor.matmul(psum_b[:], oh[:], vb[:, 512:1024], start=st, stop=sp)

            # counts: reduce cnt_acc over partitions with a single matmul
            ones128b = cp.tile([128, 1], bf16)
            nc.vector.memset(ones128b[:], 1.0)
            cnt_bf = sb.tile([128, HW], bf16)
            nc.vector.tensor_copy(cnt_bf[:], cnt_acc[:])
            psum_c = pp.tile([HW, 1], f32, tag="pc")
            nc.tensor.matmul(psum_c[:], cnt_bf[:], ones128b[:], start=True, stop=True)

            sums_sb = sb.tile([HW, BC + 1], f32)
            nc.any.tensor_copy(sums_sb[:, 0:512], psum_a[:])
            nc.any.tensor_copy(sums_sb[:, 512:1024], psum_b[:])
            nc.any.tensor_copy(sums_sb[:, 1024:1025], psum_c[:])

            # ---- exchange partials: core i receives rows for bins [8i, 8i+8)
            # from every core, interleaved as [src_core, 8] x (BC+1) ----
            nc.sync.dma_start(a2a_in.ap(), sums_sb[:])
            nc.gpsimd.collective_compute(
                "AllToAll", mybir.AluOpType.bypass, replica_groups=[cores],
                ins=[a2a_in.ap().opt()], outs=[a2a_out.ap().opt()],
            )
            red64 = sb.tile([HW, BC + 1], f32)
            nc.sync.dma_start(red64[:], a2a_out.ap())

            # reduce the 8 partials: pr[h] = sum_p sel[p, h] * red64[p]
            pr_a = pp.tile([HB, 512], f32, tag="pa")
            pr_b = pp.tile([HB, 512], f32, tag="pb")
            pr_c = pp.tile([HB, 1], f32, tag="pc")
            nc.tensor.matmul(pr_a[:], sel_sb[:], red64[:, 0:512], start=True, stop=True)
            nc.tensor.matmul(pr_b[:], sel_sb[:], red64[:, 512:1024], start=True, stop=True)
            nc.tensor.matmul(pr_c[:], sel_sb[:], red64[:, 1024:1025], start=True, stop=True)

            # ---- means for the owned 8 bins ----
            cnt = sb.tile([HB, 1], f32)
            nc.vector.tensor_scalar(cnt[:], pr_c[:], 1.0, None, amax)
            rec = sb.tile([HB, 1], f32)
            nc.vector.reciprocal(rec[:], cnt[:])
            means = sb.tile([HB, BC], f32)
            nc.vector.tensor_scalar(means[:, 0:512], pr_a[:], rec[:], None, mult)
            nc.vector.tensor_scalar(means[:, 512:1024], pr_b[:], rec[:], None, mult)

            # ---- transpose means into Linear lhsT layout (bf16) ----
            # lhst[:, kk*4+b4] column p holds means[h, b4*256 + ch*128 + p]
            # for kk = h*2 + ch; matches W rows kk*128 .. kk*128+128.
            lhst = sb.tile([128, KK * B], bf16)
            for b4 in range(B):
                for ch in range(2):
                    pt = ppt.tile([128, HB], f32)
                    lo = b4 * C + ch * 128
                    nc.tensor.transpose(pt[:], means[:, lo:lo + 128], id8[:])
                    dst = lhst[:].rearrange("p (h q) -> p h q", q=8)[:, :, 4 * ch + b4]
                    nc.any.tensor_copy(dst, pt[:])

            # ---- per-core partial Linear + bias/8, then AllReduce ----
            w_bf3 = w_bf[:].rearrange("p (kk o) -> p kk o", kk=KK)
            psum_o = pp.tile([B, OUT], f32)
            for kk in range(KK):
                nc.tensor.matmul(psum_o[:], lhst[:, kk * B:(kk + 1) * B],
                                 w_bf3[:, kk, :], start=(kk == 0), stop=False)
            nc.tensor.matmul(psum_o[:], ones_b[:], b_sb[:], start=False, stop=True)
            out_sb = sb.tile([B, OUT], f32)
            nc.any.tensor_copy(out_sb[:], psum_o[:])
            nc.sync.dma_start(ar_in.ap(), out_sb[:])
            nc.gpsimd.collective_compute(
                "AllReduce", add, replica_groups=[cores],
                ins=[ar_in.ap().opt()], outs=[ar_out.ap().opt()],
            )
            nc.sync.dma_start(out_ext.ap(), ar_out.ap())

    nc.compile()
    return nc


def _get_nc():
    global _NC
    if _NC is None:
        _NC = _build()
    return _NC


def _shard(values, coords, W, b):
    values = np.ascontiguousarray(values, dtype=np.float32)
    coords = np.ascontiguousarray(coords, dtype=np.float32)
    W = np.ascontiguousarray(W, dtype=np.float32)
    b = np.ascontiguousarray(b, dtype=np.float32)
    in_maps = []
    for i in range(N_CORES):
        in_maps.append({
            "values": np.ascontiguousarray(values[:, i * NS:(i + 1) * NS, :]),
            "coords": np.ascontiguousarray(coords[i * NS:(i + 1) * NS]),
            "W": np.ascontiguousarray(W[i * HB * C:(i + 1) * HB * C]),
            "b": b,
        })
    return in_maps


def kernel(values, coords, W, b):
    nc = _get_nc()
    in_maps = _shard(values, coords, W, b)
    res = run_bass_kernel_spmd(nc, in_maps, core_ids=list(range(N_CORES)))
    return np.asarray(res.results[0]["out"], dtype=np.float32)
